# revision 29
# baseline (speedup 1.0000x reference)
"""Multi-head attention with QK-LayerNorm on 8 TRN2 NeuronCores.

Shapes: B=2, T=2048, E=1024, H=16 heads, S=64 head dim.
Sharding: core c handles batch c//4 and the 4 heads [ (c%4)*4 , (c%4)*4+4 ).
Each core computes a partial output (its heads' contribution through Wo);
the host sums the 4 partials per batch and adds bo.

Device-side layout: activations are kept transposed ([feature, t]) so every
matmul contracts over the partition axis without on-device transposes:
  QT/KT   [s(64)*2heads = 128p, T]   (2 tiles per core, 2 heads each)
  V       [t 128p, head, s+1]        (extra ones-column -> softmax row sums)
  scores  S^T [t_k 128p, t_q 512]    (strictly-causal upper blocks skipped)
LayerNorm over s (the partition axis of QT) is done via matmul statistics
(block-diagonal ones lhsT), row math on [16, T] tiles, and a DRAM-roundtrip
partition-broadcast of the per-(head,t) scale/shift rows.
Softmax needs no max-subtraction: LN bounds logits to |q.k| <= ~2.
"""

import json
import math

import numpy as np
import ml_dtypes

import concourse.bass as bass
import concourse.bass2jax as bass2jax
import concourse.bass_utils as bass_utils
import concourse.tile as tile
from concourse import mybir
from concourse.vector_clock import ScopedClock

B, T, E, H, S = 2, 2048, 1024, 16, 64
HPC = 4            # heads per core
EPC = HPC * S      # feature cols per core = 256
LN_EPS = 1e-5
INV4 = float(E) ** -0.25
FP32 = mybir.dt.float32
BF16 = mybir.dt.bfloat16
BF = ml_dtypes.bfloat16

# ---------------------------------------------------------------------------
# Compile hook: this toolchain's walrus accepts at most ONE semaphore wait per
# TPB instruction. Tile attaches several. Split extras into standalone
# EventSemaphore (wait-only) instructions on the same engine.
# ---------------------------------------------------------------------------
_TPB_ENGINES = ("Pool", "Activation", "PE", "DVE", "SP")


def _split_multiwaits(bir_json: bytes) -> bytes:
    d = json.loads(bir_json)
    n_split = 0
    for fn in d.get("functions", []):
        for blk in fn.get("blocks", []):
            insts = blk.get("instructions", [])
            out = []
            for inst in insts:
                si = inst.get("sync_info")
                waits = (si or {}).get("on_wait") or []
                if si and len(waits) > 1 and inst.get("engine") in _TPB_ENGINES:
                    for i, w in enumerate(waits[:-1]):
                        out.append({
                            "debug": inst.get("debug", 0),
                            "engine": inst["engine"],
                            "ins": [],
                            "name": f"{inst['name']}-ws{i}",
                            "opcode": "EventSemaphore",
                            "outs": [],
                            "sync_info": {"on_update": [], "on_wait": [w]},
                        })
                        n_split += 1
                    si["on_wait"] = [waits[-1]]
                out.append(inst)
            blk["instructions"] = out
    return json.dumps(d).encode()


_orig_compile_bir_kernel = bass_utils.compile_bir_kernel


def _patched_compile_bir_kernel(bir_json, tmpdir, neff_name="file.neff"):
    return _orig_compile_bir_kernel(_split_multiwaits(bir_json), tmpdir, neff_name)


bass_utils.compile_bir_kernel = _patched_compile_bir_kernel
bass2jax.compile_bir_kernel = _patched_compile_bir_kernel


def _patched_drain_and_barrier(self, tick_clock, wait_clock):
    # Same as TileContext._drain_and_barrier but the drain's waits are emitted
    # as single-wait instructions (walrus limit).
    gc = tick_clock.global_clock
    ticks = eval(str(gc).replace("VectorClock(", "").rstrip(")"))
    sems = wait_clock.sems.allocated()
    for proc_idx, sem in sems.items():
        t = ticks[proc_idx]
        if t > 0:
            mult = 16 if proc_idx >= 11 else 1
            self.nc.sync.wait_ge(sem, t * mult)
    self.nc.sync.drain()
    self.nc.all_engine_barrier()
    assert self.sems is not None
    popped = self.nc._tile_sem_poison_stack.pop()
    assert popped is self._sem_poison
    self.nc.clear_and_free_semaphores(list(self.sems.allocated().values()))
    self.nc.all_engine_barrier()


tile.TileContext._drain_and_barrier = _patched_drain_and_barrier


# ---------------------------------------------------------------------------
# Device kernel (identical program on all 8 cores)
# ---------------------------------------------------------------------------


def _act_raw(nc, out, in_, func):
    # nc.scalar.activation refuses Reciprocal (accuracy); our tolerance is
    # 2e-2 so the LUT version is fine. Emit InstActivation directly.
    eng = nc.scalar
    inputs = [eng.lower_ap(in_)]
    for arg in (0.0, 1.0, 0.0):  # bias, scale, alpha
        inputs.append(mybir.ImmediateValue(dtype=mybir.dt.float32, value=arg))
    return eng.add_instruction(
        mybir.InstActivation(
            name=nc.get_next_instruction_name(),
            func=func,
            ins=inputs,
            outs=[eng.lower_ap(out)],
        )
    )


def _build_bass():
    nc = bass.Bass()
    xtq_e = nc.dram_tensor("xtq", [E, T], BF16, kind="ExternalInput")
    xtk_e = nc.dram_tensor("xtk", [E, T], BF16, kind="ExternalInput")
    xtv_e = nc.dram_tensor("xtv", [E, T], BF16, kind="ExternalInput")
    wq_e = nc.dram_tensor("wq", [E, EPC], BF16, kind="ExternalInput")
    wk_e = nc.dram_tensor("wk", [E, EPC], BF16, kind="ExternalInput")
    wv_e = nc.dram_tensor("wv", [E, EPC], BF16, kind="ExternalInput")
    wo_e = nc.dram_tensor("wo", [EPC, E], BF16, kind="ExternalInput")
    masks_e = nc.dram_tensor("masks", [128, 4, 1024], BF16, kind="ExternalInput")
    eye_e = nc.dram_tensor("eye2", [128, 2], BF16, kind="ExternalInput")
    wb_e = nc.dram_tensor("wbcols", [128, 4], FP32, kind="ExternalInput")
    selrep_e = nc.dram_tensor("selrep", [128, 128], BF16, kind="ExternalInput")
    selh_e = nc.dram_tensor("selh", [2, 128], BF16, kind="ExternalInput")
    out_e = nc.dram_tensor("out", [T, E], FP32, kind="ExternalOutput")

    xtq = xtq_e.ap().rearrange("(o p) t -> p o t", p=128)   # [128, 8, T]
    xtk = xtk_e.ap().rearrange("(o p) t -> p o t", p=128)
    xtv = xtv_e.ap().rearrange("(o p) t -> p o t", p=128)
    wq_a = wq_e.ap().rearrange("(o p) f -> p o f", p=128)   # [128, 8, 256]
    wk_a = wk_e.ap().rearrange("(o p) f -> p o f", p=128)
    wv_a = wv_e.ap().rearrange("(o p) f -> p o f", p=128)
    wo_a = wo_e.ap().rearrange("(o p) f -> p o f", p=128)   # [128, 2, 1024]

    with tile.TileContext(nc) as tc:
        with tc.tile_pool(name="singles", bufs=1) as singles, \
             tc.tile_pool(name="xstream", bufs=4) as xstream, \
             tc.tile_pool(name="work", bufs=2) as work, \
             tc.tile_pool(name="rows", bufs=1) as rows, \
             tc.tile_pool(name="expp", bufs=8) as expp, \
             tc.tile_pool(name="outp", bufs=3) as outp, \
             tc.tile_pool(name="otsbp", bufs=5) as otsbp, \
             tc.tile_pool(name="rcp", bufs=10) as rcpp, \
             tc.tile_pool(name="rbp", bufs=3) as rbp, \
             tc.tile_pool(name="psu", bufs=2, space="PSUM") as psu, \
             tc.tile_pool(name="psu1", bufs=4, space="PSUM") as psu1:

            # ---- resident constants (issue order = DMA priority) ---------
            wq_sb = singles.tile([128, 8, EPC], BF16)
            nc.sync.dma_start(out=wq_sb, in_=wq_a)
            wk_sb = singles.tile([128, 8, EPC], BF16)
            eye_sb = singles.tile([128, 2], BF16)
            nc.sync.dma_start(out=eye_sb, in_=eye_e.ap())
            wb_sb = singles.tile([128, 4], FP32)
            nc.sync.dma_start(out=wb_sb, in_=wb_e.ap())
            selrep_sb = singles.tile([128, 128], BF16)
            nc.sync.dma_start(out=selrep_sb, in_=selrep_e.ap())
            selh0_sb = singles.tile([1, 128], BF16)
            nc.sync.dma_start(out=selh0_sb, in_=selh_e.ap()[0:1, :])
            selh1_sb = singles.tile([1, 128], BF16)
            nc.sync.dma_start(out=selh1_sb, in_=selh_e.ap()[1:2, :])
            xtv_sb = singles.tile([128, 8, T], BF16)
            wv_sb = singles.tile([128, 8, EPC], BF16)
            masks_sb = singles.tile([128, 4, 1024], BF16)
            wo_sb = singles.tile([128, 2, E], BF16)

            qt = [singles.tile([128, T], BF16, tag=f"qt{m}", name=f"qt{m}") for m in range(2)]
            kt = [singles.tile([128, T], BF16, tag=f"kt{m}", name=f"kt{m}") for m in range(2)]
            vhat = singles.tile([128, 16, HPC, S + 1], BF16)
            otb = [singles.tile([128, T], BF16, tag=f"otb{m}", name=f"otb{m}") for m in range(2)]
            nc.vector.memset(vhat[:, :, :, S:S + 1], 1.0)

            # ---- Q/K projections + LN statistics (interleaved) -----------
            sums_t = rows.tile([128, T], FP32)
            sumsq_t = rows.tile([128, T], FP32)

            def ln_stats(src_t, m, c):
                sq = work.tile([128, T], BF16, tag="sq")
                nc.vector.tensor_tensor(out=sq, in0=src_t[m], in1=src_t[m],
                                        op=mybir.AluOpType.mult)
                for n in range(4):
                    sl = slice(n * 512, (n + 1) * 512)
                    ps_s = psu1.tile([128, 512], FP32, tag="u1", name="st_s")
                    ps_q = psu1.tile([128, 512], FP32, tag="u1", name="st_q")
                    nc.tensor.matmul(ps_s[0:2, :], lhsT=eye_sb, rhs=src_t[m][:, sl],
                                     start=True, stop=True)
                    nc.tensor.matmul(ps_q[0:2, :], lhsT=eye_sb, rhs=sq[:, sl],
                                     start=True, stop=True)
                    if n % 2 == 0:
                        nc.scalar.activation(out=sums_t[32 * c:32 * c + 2, sl],
                                             in_=ps_s[0:2, :],
                                             func=mybir.ActivationFunctionType.Copy)
                        nc.scalar.activation(out=sumsq_t[32 * c:32 * c + 2, sl],
                                             in_=ps_q[0:2, :],
                                             func=mybir.ActivationFunctionType.Copy)
                    else:
                        nc.vector.tensor_copy(out=sums_t[32 * c:32 * c + 2, sl],
                                              in_=ps_s[0:2, :])
                        nc.vector.tensor_copy(out=sumsq_t[32 * c:32 * c + 2, sl],
                                              in_=ps_q[0:2, :])

            for qk_i, (x_ap, w_sb, dst) in enumerate(((xtq, wq_sb, qt), (xtk, wk_sb, kt))):
                if qk_i == 1:
                    nc.sync.dma_start(out=wk_sb, in_=wk_a)
                for m in range(2):
                    if qk_i == 1 and m == 1:
                        nc.sync.dma_start(out=xtv_sb, in_=xtv)
                        nc.sync.dma_start(out=wv_sb, in_=wv_a)
                        nc.sync.dma_start(out=masks_sb, in_=masks_e.ap())
                        nc.sync.dma_start(out=wo_sb, in_=wo_a)
                    pss = [psu.tile([128, 1024], FP32, tag="u", name=f"pss{j}")
                           for j in range(2)]
                    for e8 in range(8):
                        xc = xstream.tile([128, T], BF16, tag="xchunk")
                        nc.sync.dma_start(out=xc, in_=x_ap[:, e8, :])
                        for n in range(4):
                            nc.tensor.matmul(
                                pss[n // 2][:, (n % 2) * 512:(n % 2) * 512 + 512],
                                lhsT=w_sb[:, e8, m * 128:(m + 1) * 128],
                                rhs=xc[:, n * 512:(n + 1) * 512],
                                start=(e8 == 0), stop=(e8 == 7))
                    for j in range(2):
                        nc.vector.tensor_copy(
                            out=dst[m][:, j * 1024:(j + 1) * 1024], in_=pss[j])
                    ln_stats(dst, m, 2 * qk_i + m)

            # ---- LN row math (overlaps V projection) --------------------
            eps_col = singles.tile([128, 1], FP32)
            nc.vector.memset(eps_col, LN_EPS)
            nc.vector.tensor_scalar_mul(sums_t, sums_t, 1.0 / S)          # mu
            nc.vector.tensor_scalar_mul(sumsq_t, sumsq_t, 1.0 / S)
            tmp = rows.tile([128, T], FP32)
            nc.vector.tensor_tensor(out=tmp, in0=sums_t, in1=sums_t,
                                    op=mybir.AluOpType.mult)
            nc.vector.tensor_tensor(out=sumsq_t, in0=sumsq_t, in1=tmp,
                                    op=mybir.AluOpType.subtract)
            nc.vector.tensor_scalar_max(sumsq_t, sumsq_t, 0.0)
            nc.scalar.activation(out=sumsq_t, in_=sumsq_t,
                                 func=mybir.ActivationFunctionType.Sqrt,
                                 bias=eps_col)
            _act_raw(nc, sumsq_t, sumsq_t,
                     mybir.ActivationFunctionType.Reciprocal)             # rstd
            nc.vector.tensor_tensor(out=tmp, in0=sums_t, in1=sumsq_t,
                                    op=mybir.AluOpType.mult)              # mu*rstd
            c_bfrows = rows.tile([128, T], BF16)
            a_bfrows = rows.tile([128, T], BF16)
            nc.vector.tensor_copy(out=c_bfrows, in_=tmp)
            nc.vector.tensor_copy(out=a_bfrows, in_=sumsq_t)

            # ---- V projection (natural layout + ones column) -------------
            for t16 in range(16):
                psv = psu.tile([128, 1024], FP32, tag="u", name="psv")
                for e8 in range(8):
                    nc.tensor.matmul(
                        psv[:, 0:EPC], lhsT=xtv_sb[:, e8, t16 * 128:(t16 + 1) * 128],
                        rhs=wv_sb[:, e8, :], start=(e8 == 0), stop=(e8 == 7))
                nc.scalar.activation(
                    out=vhat[:, t16, :, 0:S],
                    in_=psv[:, 0:EPC].rearrange("p (h s) -> p h s", h=HPC),
                    func=mybir.ActivationFunctionType.Copy)

            # ---- LN apply via PE row-broadcast --------------------------
            # bp[:, 0:512] = a-row broadcast, bp[:, 512:1024] = c-row; the
            # selector lhsT lives at the same 32-aligned base as the rows.
            def ln_apply(src_t, m, c):
                sel = selrep_sb[32 * c:32 * c + 2, :]
                wcol = wb_sb[:, 0:1] if src_t is qt else wb_sb[:, 2:3]
                bcol = wb_sb[:, 1:2] if src_t is qt else wb_sb[:, 3:4]
                for ch in range(4):
                    sl = slice(ch * 512, (ch + 1) * 512)
                    bpa = psu1.tile([128, 512], FP32, tag="u1", name="bpa")
                    bpc = psu1.tile([128, 512], FP32, tag="u1", name="bpc")
                    nc.tensor.matmul(bpa, lhsT=sel,
                                     rhs=a_bfrows[32 * c:32 * c + 2, sl],
                                     start=True, stop=True,
                                     tile_position=(32 * c, 0))
                    nc.tensor.matmul(bpc, lhsT=sel,
                                     rhs=c_bfrows[32 * c:32 * c + 2, sl],
                                     start=True, stop=True,
                                     tile_position=(32 * c, 0))
                    nc.vector.tensor_tensor(out=src_t[m][:, sl], in0=src_t[m][:, sl],
                                            in1=bpa,
                                            op=mybir.AluOpType.mult)
                    nc.vector.tensor_tensor(out=src_t[m][:, sl], in0=src_t[m][:, sl],
                                            in1=bpc,
                                            op=mybir.AluOpType.subtract)
                nc.vector.tensor_scalar(out=src_t[m], in0=src_t[m],
                                        scalar1=wcol, scalar2=bcol,
                                        op0=mybir.AluOpType.mult,
                                        op1=mybir.AluOpType.add)

            # ---- attention (two head-pair streams interleaved) -----------
            ln_apply(qt, 0, 0)
            ln_apply(kt, 0, 2)
            ln_apply(qt, 1, 1)
            ln_apply(kt, 1, 3)

            def finish_norm(state):
                m_, qb_, otsb_, rcs_ = state
                nb = psu1.tile([128, 512], FP32, tag="u1", name="nb")
                nc.tensor.matmul(nb, lhsT=selh0_sb, rhs=rcs_[0],
                                 start=True, stop=False)
                nc.tensor.matmul(nb, lhsT=selh1_sb, rhs=rcs_[1],
                                 start=False, stop=True)
                rb = rbp.tile([128, 512], FP32, tag="rb")
                nc.vector.reciprocal(out=rb, in_=nb)
                nc.vector.tensor_tensor(
                    out=otb[m_][:, qb_ * 512:(qb_ + 1) * 512],
                    in0=otsb_, in1=rb, op=mybir.AluOpType.mult)

            norm_pending = []
            for qb in range(4):
                otps = {}
                for m in range(2):
                    otps[m] = [psu1.tile([128, 512], FP32, tag="u1",
                                         name=f"otp{m}{h_}") for h_ in range(2)]
                nkb = 4 * qb + 4
                ex_prev = {}
                for kb in range(nkb):
                    for m in range(2):
                        st = psu.tile([128, 1024], FP32, tag="u", name="st")
                        for h in range(2):
                            pa = slice(64 * h, 64 * h + 64)
                            nc.tensor.matmul(
                                st[:, h * 512:(h + 1) * 512],
                                lhsT=kt[m][pa, kb * 128:(kb + 1) * 128],
                                rhs=qt[m][pa, qb * 512:(qb + 1) * 512],
                                start=True, stop=True)
                        ex = expp.tile([128, 1024], BF16, tag="exp")
                        nc.scalar.activation(
                            out=ex, in_=st,
                            func=mybir.ActivationFunctionType.Exp)
                        d = kb - 4 * qb
                        if d >= 0:  # diagonal block: causal 0/1 mask
                            nc.gpsimd.tensor_tensor(
                                out=ex, in0=ex, in1=masks_sb[:, d, :],
                                op=mybir.AluOpType.mult)
                        if kb > 0:
                            exp_, kb_ = ex_prev[m]
                            for h in range(2):
                                nc.tensor.matmul(
                                    otps[m][h][0:S + 1, :],
                                    lhsT=vhat[:, kb_, 2 * m + h, :],
                                    rhs=exp_[:, h * 512:(h + 1) * 512],
                                    start=(kb_ == 0), stop=False)
                        ex_prev[m] = (ex, kb)
                    if kb == 1 and norm_pending:
                        for state in norm_pending:
                            finish_norm(state)
                        norm_pending = []
                for m in range(2):
                    exp_, kb_ = ex_prev[m]
                    for h in range(2):
                        nc.tensor.matmul(
                            otps[m][h][0:S + 1, :],
                            lhsT=vhat[:, kb_, 2 * m + h, :],
                            rhs=exp_[:, h * 512:(h + 1) * 512],
                            start=(kb_ == 0), stop=True)
                # evict O^T + sums rows now (frees the accumulators); the
                # remaining normalize math runs during the next qb's loop
                for m in range(2):
                    otsb = otsbp.tile([128, 512], FP32, tag="otsb")
                    rcs = []
                    for h in range(2):
                        rc = rcpp.tile([1, 512], BF16, tag="rc", name=f"rc{h}")
                        nc.vector.tensor_copy(out=rc, in_=otps[m][h][S:S + 1, :])
                        nc.vector.tensor_copy(out=otsb[64 * h:64 * h + 64, :],
                                              in_=otps[m][h][0:S, :])
                        rcs.append(rc)
                    norm_pending.append((m, qb, otsb, rcs))
            for state in norm_pending:
                finish_norm(state)

            # ---- output projection --------------------------------------
            for t16 in range(16):
                pso = psu.tile([128, 1024], FP32, tag="u", name="pso")
                for e2 in range(2):
                    for m in range(2):
                        nc.tensor.matmul(
                            pso[:, e2 * 512:(e2 + 1) * 512],
                            lhsT=otb[m][:, t16 * 128:(t16 + 1) * 128],
                            rhs=wo_sb[:, m, e2 * 512:(e2 + 1) * 512],
                            start=(m == 0), stop=(m == 1))
                osb = outp.tile([128, 1024], FP32, tag="osb")
                nc.vector.tensor_copy(out=osb, in_=pso)
                nc.sync.dma_start(
                    out=out_e.ap()[t16 * 128:(t16 + 1) * 128, :],
                    in_=osb)
    return nc


_NC_CACHE = None


def _get_nc():
    global _NC_CACHE
    if _NC_CACHE is None:
        _NC_CACHE = _build_bass()
    return _NC_CACHE


# ---------------------------------------------------------------------------
# Host wrapper
# ---------------------------------------------------------------------------

def _make_masks():
    # mask[p, d_idx, f] = 1.0 if p + d <= f else 0, d = 128*d_idx
    p = np.arange(128)[:, None, None]
    dd = (np.arange(4) * 128)[None, :, None]
    f = np.arange(512)[None, None, :]
    m = ((p + dd) <= f).astype(BF)           # [128, 4, 512]
    return np.concatenate([m, m], axis=2)    # [128, 4, 1024] (2 head halves)


def kernel(queries, keys, values, Wq, Wk, Wv, Wo, bo, q_ln_w, q_ln_b,
           k_ln_w, k_ln_b):
    from concourse.bass_utils import run_bass_kernel_spmd

    nc = _get_nc()

    masks = _make_masks()
    eye2 = np.zeros((128, 2), dtype=BF)
    eye2[0:64, 0] = 1
    eye2[64:128, 1] = 1
    selrep = np.zeros((128, 128), dtype=BF)
    for c in range(4):
        selrep[32 * c, 0:64] = 1
        selrep[32 * c + 1, 64:128] = 1
    selh = np.zeros((2, 128), dtype=BF)
    selh[0, 0:64] = 1
    selh[1, 64:128] = 1
    wb = np.stack([
        np.tile(np.asarray(q_ln_w, np.float32) * INV4, 2),
        np.tile(np.asarray(q_ln_b, np.float32) * INV4, 2),
        np.tile(np.asarray(k_ln_w, np.float32) * INV4, 2),
        np.tile(np.asarray(k_ln_b, np.float32) * INV4, 2),
    ], axis=1).astype(np.float32)

    in_maps = []
    for core in range(8):
        b = core // 4
        cs = (core % 4) * EPC
        sl = slice(cs, cs + EPC)
        in_maps.append({
            "xtq": np.ascontiguousarray(np.asarray(queries[b], np.float32).T).astype(BF),
            "xtk": np.ascontiguousarray(np.asarray(keys[b], np.float32).T).astype(BF),
            "xtv": np.ascontiguousarray(np.asarray(values[b], np.float32).T).astype(BF),
            "wq": np.ascontiguousarray(np.asarray(Wq, np.float32)[:, sl]).astype(BF),
            "wk": np.ascontiguousarray(np.asarray(Wk, np.float32)[:, sl]).astype(BF),
            "wv": np.ascontiguousarray(np.asarray(Wv, np.float32)[:, sl]).astype(BF),
            "wo": np.ascontiguousarray(np.asarray(Wo, np.float32)[sl, :]).astype(BF),
            "masks": masks,
            "eye2": eye2,
            "wbcols": wb,
            "selrep": selrep,
            "selh": selh,
        })

    kernel._last_in_maps = in_maps
    res = run_bass_kernel_spmd(nc, in_maps, core_ids=list(range(8)))
    outs = [res.results[i]["out"] for i in range(8)]
    bo32 = np.asarray(bo, np.float32)
    full = np.stack([
        outs[0] + outs[1] + outs[2] + outs[3] + bo32,
        outs[4] + outs[5] + outs[6] + outs[7] + bo32,
    ]).astype(np.float32)
    return full
